# revision 1
# baseline (speedup 1.0000x reference)
"""TRN2 Bass kernel for nn_E59bGatedHighwayCell (T=2048, B=16, D=1024).

Strategy
--------
Data-parallel over the batch axis: 8 NeuronCores x 2 batch rows each
(one SPMD program, per-core input shards).

Per core:
  * Big matmuls Wx = x@W.T and z = x@W_g.T run as fp32 PE matmuls in
    T-chunks of 128, PSUM-accumulated over 8 K-chunks.
  * c_t = sigmoid(z + b_g) * (Wx + b) fused on ACT (sigmoid w/ bias) +
    one scalar_tensor_tensor per [128, 256] tile.
  * c is DMA-shuffled into the "scan layout": partition p = b*64 + e//16,
    free f = e%16, so the RMS mean over e becomes a per-partition
    accumulate (16 wide) + one 64x64 block-diagonal ones-matmul (weights
    0.5/D -> produces m/2 broadcast to all partitions of the block).
  * The sequential scan (2048 steps) keeps the state unnormalized:
        u_t = u_{t-1} * r_{t-1} + c_t     (one STT, r is per-partition)
        persum = per-partition sum of u_t^2  (STT with accum_out)
        mh = blockdiag-matmul(persum)        (two 64x64 tile_position MMs)
        r_t = rsqrt(2*mh)                    (2 custom DVE ops: linear
                                              seed + 3 Newton passes)
    eps (1e-6) is folded out: relative effect <= eps/(2m) ~ 6e-7.
  * h_t is recovered in batch per chunk: h_{t-1} = u_t - c_t (plus one
    tensor_scalar for the chunk's last h), outs = h^2 * sigmoid(h) on
    ACT+DVE, both DMA'd out per chunk.

All ACT usage is from the sigmoid_and_others table set (sigmoid/square)
so there are no ACT table reloads; rsqrt lives on the Vector engine as
two custom-DVE instructions.
"""

import sys

if "/opt/trn_rl_repo" not in sys.path:
    sys.path.insert(0, "/opt/trn_rl_repo")

from contextlib import ExitStack

import numpy as np

import concourse.bass as bass  # noqa: F401  (AP types)
import concourse.tile as tile
from concourse import bacc, mybir
from concourse import dve_ops
from concourse.dve_spec import C0, C1, C2, Spec, Src0, Src1, lower, sq, _has_src1
from concourse.dve_uop import DveOpSpec
from concourse.bass_utils import run_bass_kernel_spmd

F32 = mybir.dt.float32
AF = mybir.ActivationFunctionType
OP = mybir.AluOpType

D = 1024
B2 = 2
KC = 8
MC = 8
N_CORES = 8
T_FULL = 2048
TC = 128

# ---------------- custom DVE rsqrt ----------------
# rsqrt(2*mh) via linear minimax seed on mh in [0.4, 0.75] (m in
# [0.8, 1.5]; measured m range for this problem is [0.93, 1.14]) plus
# three Newton passes across two DVE instructions. <4e-9 rel error for
# m in [0.7, 1.6], graceful degradation outside.
SEED_C0 = 1.409402769345198
SEED_C1 = -0.8274676059420824
NR_C2 = 1.5


def _pin_sha(name, spec):
    shas = {}
    for ver in ("v3", "v4"):
        try:
            s = DveOpSpec(
                name=name,
                opcode=dve_ops._SUB_OPCODE_FOR_NAME[name],
                uops=lower(spec, ver=ver),
                rd1_en=_has_src1(spec),
            )
            shas[ver] = s.sha(ver)
        except Exception:
            pass
    return shas


def _register(name, spec):
    if name in dve_ops._SUB_OPCODE_FOR_NAME:
        return next(o for o in dve_ops.OPS if o.name == name)
    dve_ops._SUB_OPCODE_FOR_NAME[name] = dve_ops._CUSTOM_DVE_ROW_BASE + len(
        dve_ops.OPS
    )
    op = dve_ops.DveOp(name, spec, subdim=False, uops_sha=_pin_sha(name, spec))
    dve_ops.OPS.append(op)
    dve_ops.CUSTOM_DVE_SPECS[name] = spec
    return op


def _ref_seed_nr1(in0, in1, c0, c1, c2):
    in0 = in0.astype(np.float32)
    y0 = (c0 + in0 * c1).astype(np.float32)
    return (y0 * (c2 - in0 * y0 * y0)).astype(np.float32)


def _ref_nr2(in0, in1, c0, c1, c2):
    in0 = in0.astype(np.float32)
    y = in1.astype(np.float32)
    y = (y * (c2 - in0 * y * y)).astype(np.float32)
    return (y * (c2 - in0 * y * y)).astype(np.float32)


_y0 = C0 + Src0 * C1
RSQRT_SEED_NR1 = _register(
    "RSQRT_SEED_NR1", Spec(body=_y0 * (C2 - Src0 * sq(_y0)), reference=_ref_seed_nr1)
)
_y2 = Src1 * (C2 - Src0 * sq(Src1))
RSQRT_NR2 = _register(
    "RSQRT_NR2", Spec(body=_y2 * (C2 - Src0 * sq(_y2)), reference=_ref_nr2)
)


# ---------------- program builder ----------------


def build_nc(T=T_FULL, Tc=TC):
    nc = bacc.Bacc("TRN2", target_bir_lowering=False, debug=False)

    xT = nc.dram_tensor("xT", [KC, 128, B2, T], F32, kind="ExternalInput").ap()
    h0s = nc.dram_tensor("h0s", [128, 16], F32, kind="ExternalInput").ap()
    wT = nc.dram_tensor("wT", [KC, 128, D], F32, kind="ExternalInput").ap()
    wgT = nc.dram_tensor("wgT", [KC, 128, D], F32, kind="ExternalInput").ap()
    bias = nc.dram_tensor("bias", [MC, 128], F32, kind="ExternalInput").ap()
    biasg = nc.dram_tensor("biasg", [MC, 128], F32, kind="ExternalInput").ap()
    bd = nc.dram_tensor("bd", [128, 64], F32, kind="ExternalInput").ap()

    h_out = nc.dram_tensor("h_out", [T + 1, B2, D], F32, kind="ExternalOutput").ap()
    outs = nc.dram_tensor("outs", [T, B2, D], F32, kind="ExternalOutput").ap()

    nchunks = T // Tc
    assert T % Tc == 0

    with tile.TileContext(nc) as tc, ExitStack() as ctx:
        const = ctx.enter_context(tc.tile_pool(name="const", bufs=1))
        xp = ctx.enter_context(tc.tile_pool(name="xp", bufs=2))
        cp = ctx.enter_context(tc.tile_pool(name="cp", bufs=2))
        hp = ctx.enter_context(tc.tile_pool(name="hp", bufs=2))
        sp = ctx.enter_context(tc.tile_pool(name="sp", bufs=4))
        scr = ctx.enter_context(tc.tile_pool(name="scr", bufs=6))
        ps = ctx.enter_context(tc.tile_pool(name="ps", bufs=2, space="PSUM"))
        pss = ctx.enter_context(tc.tile_pool(name="pss", bufs=2, space="PSUM"))

        w_sb = const.tile([128, KC, D], F32, tag="w")
        wg_sb = const.tile([128, KC, D], F32, tag="wg")
        for kc in range(KC):
            nc.sync.dma_start(out=w_sb[:, kc, :], in_=wT[kc])
            nc.sync.dma_start(out=wg_sb[:, kc, :], in_=wgT[kc])
        b_sb = const.tile([128, MC], F32, tag="b")
        bg_sb = const.tile([128, MC], F32, tag="bg")
        nc.sync.dma_start(out=b_sb, in_=bias.rearrange("m p -> p m"))
        nc.sync.dma_start(out=bg_sb, in_=biasg.rearrange("m p -> p m"))
        bd_sb = const.tile([128, 64], F32, tag="bd")
        nc.sync.dma_start(out=bd_sb, in_=bd)

        u0 = const.tile([128, 16], F32, tag="u0")
        nc.sync.dma_start(out=u0, in_=h0s)
        r0 = const.tile([128, 1], F32, tag="r0")
        nc.vector.memset(r0, 1.0)
        for b in range(B2):
            nc.sync.dma_start(
                out=h_out[0, b].rearrange("(g f) -> g f", f=16),
                in_=u0[b * 64 : (b + 1) * 64, :],
            )

        u_prev = u0
        r_prev = r0

        h_view = h_out.rearrange("t b (g f) -> t b g f", f=16)
        o_view = outs.rearrange("t b (g f) -> t b g f", f=16)

        for ch in range(nchunks):
            t0 = ch * Tc

            # ---- phase A: c for this chunk ----
            x_sb = xp.tile([128, KC, B2, Tc], F32, tag="x")
            for kc in range(KC):
                nc.sync.dma_start(out=x_sb[:, kc, :, :], in_=xT[kc, :, :, t0 : t0 + Tc])
            c_scan = cp.tile([128, 16, Tc], F32, tag="c")
            for mc in range(MC):
                p_wx = ps.tile([128, Tc * B2], F32, tag="pwx")
                p_z = ps.tile([128, Tc * B2], F32, tag="pz")
                for kc in range(KC):
                    lw = w_sb[:, kc, mc * 128 : (mc + 1) * 128]
                    lg = wg_sb[:, kc, mc * 128 : (mc + 1) * 128]
                    rx = x_sb[:, kc, :, :].rearrange("p b t -> p (b t)")
                    nc.tensor.matmul(
                        p_wx, lhsT=lw, rhs=rx, start=(kc == 0), stop=(kc == KC - 1)
                    )
                    nc.tensor.matmul(
                        p_z, lhsT=lg, rhs=rx, start=(kc == 0), stop=(kc == KC - 1)
                    )
                sig = sp.tile([128, Tc * B2], F32, tag="sig")
                nc.scalar.activation(
                    out=sig, in_=p_z, func=AF.Sigmoid, bias=bg_sb[:, mc : mc + 1]
                )
                c_e = sp.tile([128, B2, Tc], F32, tag="ce")
                nc.vector.scalar_tensor_tensor(
                    out=c_e.rearrange("p b t -> p (b t)"),
                    in0=p_wx,
                    scalar=b_sb[:, mc : mc + 1],
                    in1=sig,
                    op0=OP.add,
                    op1=OP.mult,
                )
                # shuffle into scan layout (dest partition b*64+mc*8+p//16,
                # dest free (f=p%16, t))
                for b in range(B2):
                    nc.sync.dma_start(
                        out=c_scan[b * 64 + mc * 8 : b * 64 + mc * 8 + 8, :, :],
                        in_=c_e[:, b, :],
                    )

            # f-inner copy for contiguous per-step reads
            c_fi = cp.tile([128, Tc, 16], F32, tag="cfi")
            nc.vector.tensor_copy(c_fi, c_scan.rearrange("p f t -> p t f"))

            # ---- phase B: sequential scan ----
            u_hist = cp.tile([128, Tc, 16], F32, tag="u")
            r_last = None
            for t in range(Tc):
                u_new = u_hist[:, t, :]
                nc.vector.scalar_tensor_tensor(
                    out=u_new,
                    in0=u_prev,
                    scalar=r_prev[:, 0:1],
                    in1=c_fi[:, t, :],
                    op0=OP.mult,
                    op1=OP.add,
                )
                sqt = scr.tile([128, 16], F32, tag="sq")
                psum_p = scr.tile([128, 1], F32, tag="pp")
                nc.vector.scalar_tensor_tensor(
                    out=sqt,
                    in0=u_new,
                    scalar=1.0,
                    in1=u_new,
                    op0=OP.mult,
                    op1=OP.mult,
                    accum_out=psum_p,
                )
                bsum = pss.tile([128, 1], F32, tag="bs")
                nc.tensor.matmul(
                    bsum[0:64, :],
                    lhsT=bd_sb[0:64, :],
                    rhs=psum_p[0:64, :],
                    start=True,
                    stop=True,
                    tile_position=(0, 0),
                )
                nc.tensor.matmul(
                    bsum[64:128, :],
                    lhsT=bd_sb[64:128, :],
                    rhs=psum_p[64:128, :],
                    start=True,
                    stop=True,
                    tile_position=(64, 64),
                )
                q = scr.tile([128, 1], F32, tag="q")
                r_new = scr.tile([128, 1], F32, tag="r")
                nc.vector._custom_dve(
                    RSQRT_SEED_NR1,
                    out=q,
                    in0=bsum,
                    s0=SEED_C0,
                    s1=SEED_C1,
                    imm2=NR_C2,
                )
                nc.vector._custom_dve(
                    RSQRT_NR2, out=r_new, in0=bsum, in1=q, imm2=NR_C2
                )
                u_prev = u_new
                r_prev = r_new
                r_last = r_new

            # ---- phase C: recover h, compute outs, DMA out ----
            h_ch = hp.tile([128, Tc, 16], F32, tag="h")
            nc.vector.tensor_tensor(
                out=h_ch[:, 0 : Tc - 1, :],
                in0=u_hist[:, 1:Tc, :],
                in1=c_fi[:, 1:Tc, :],
                op=OP.subtract,
            )
            nc.vector.tensor_scalar_mul(
                out=h_ch[:, Tc - 1, :], in0=u_hist[:, Tc - 1, :], scalar1=r_last[:, 0:1]
            )
            hflat = h_ch.rearrange("p t f -> p (t f)")
            sgh = hp.tile([128, Tc * 16], F32, tag="sgh")
            nc.scalar.activation(out=sgh, in_=hflat, func=AF.Sigmoid)
            hsq = hp.tile([128, Tc * 16], F32, tag="hsq")
            nc.scalar.activation(out=hsq, in_=hflat, func=AF.Square)
            o_ch = hp.tile([128, Tc, 16], F32, tag="o")
            nc.vector.tensor_tensor(
                out=o_ch.rearrange("p t f -> p (t f)"), in0=sgh, in1=hsq, op=OP.mult
            )
            for b in range(B2):
                nc.sync.dma_start(
                    out=h_view[t0 + 1 : t0 + Tc + 1, b].rearrange("t g f -> g t f"),
                    in_=h_ch[b * 64 : (b + 1) * 64, :, :],
                )
                nc.sync.dma_start(
                    out=o_view[t0 : t0 + Tc, b].rearrange("t g f -> g t f"),
                    in_=o_ch[b * 64 : (b + 1) * 64, :, :],
                )

    nc.compile()
    return nc


# ---------------- host side ----------------

_NC_CACHE = {}


def _get_nc():
    key = (T_FULL, TC)
    if key not in _NC_CACHE:
        _NC_CACHE[key] = build_nc(*key)
    return _NC_CACHE[key]


def shard_inputs(x, h0, W, W_g, b, b_g, T=T_FULL):
    x = np.ascontiguousarray(np.asarray(x, np.float32))
    h0 = np.asarray(h0, np.float32)
    W = np.asarray(W, np.float32)
    W_g = np.asarray(W_g, np.float32)
    b = np.asarray(b, np.float32)
    b_g = np.asarray(b_g, np.float32)

    wT = np.ascontiguousarray(W.T.reshape(KC, 128, D))
    wgT = np.ascontiguousarray(W_g.T.reshape(KC, 128, D))
    bias = np.ascontiguousarray(b.reshape(MC, 128))
    biasg = np.ascontiguousarray(b_g.reshape(MC, 128))
    bd = np.full((128, 64), 0.5 / D, np.float32)

    in_maps = []
    for k in range(N_CORES):
        xs = x[:, B2 * k : B2 * (k + 1), :]
        xT = np.ascontiguousarray(xs.transpose(2, 1, 0)).reshape(KC, 128, B2, T)
        h0s = np.ascontiguousarray(h0[B2 * k : B2 * (k + 1)].reshape(128, 16))
        in_maps.append(
            dict(xT=xT, h0s=h0s, wT=wT, wgT=wgT, bias=bias, biasg=biasg, bd=bd)
        )
    return in_maps


def kernel(x, h0, W, W_g, b, b_g):
    """Full inputs in, full outputs out (outs [T,B,D], h [T+1,B,D])."""
    T = x.shape[0]
    assert (T, x.shape[1], x.shape[2]) == (T_FULL, 16, D), x.shape
    nc = _get_nc()
    in_maps = shard_inputs(x, h0, W, W_g, b, b_g, T=T)
    res = run_bass_kernel_spmd(nc, in_maps, core_ids=list(range(N_CORES)))
    outs = np.empty((T, 16, D), np.float32)
    h = np.empty((T + 1, 16, D), np.float32)
    for k in range(N_CORES):
        outs[:, B2 * k : B2 * (k + 1), :] = res.results[k]["outs"]
        h[:, B2 * k : B2 * (k + 1), :] = res.results[k]["h_out"]
    return outs, h
